# revision 1
# baseline (speedup 1.0000x reference)
"""Trainium2 Bass kernel for InstructedAttentionPositionScores.

Computes the [1, H, Q, K] attention bias of the reference nn.Module.
Sharding: one head per NeuronCore (8 heads, 8 cores, tensor parallel).

Structure of the per-head [Q, K] output (Q = K = 4708, dim_i = 100):
  rows 0..99                       "instruction" rows
    cols 0..99   : inst block (block-diag intra/inter einsum scores)
    cols 100..   : cic[row] broadcast along columns
  rows 100..4707                   "content" rows (N = 24*24*8 = 4608)
    cols 0..99   : cci[col] broadcast along rows (every row identical)
    cols 100..   : content[i, j] = (rs[hi,hj] + cs[wi,wj] + ds[di,dj]) / 3
                   with i = hi*192 + wi*8 + di  (and same for j)

All einsums are tiny (<=10 MFLOP total) and are done on host in float64;
the device kernel does the memory-bound expansion to 8 x 88.7 MB:
  content[i, j] = cd[i % 192, j % 192] + rs[i // 192, j // 192]
where cd[a, b] = cs[a//8, b//8] + ds[a%8, b%8] is a [192, 192] pattern
kept resident in SBUF (replicated to a [384, 4608] tiled form so every
128-row output tile reads one aligned slice), and the rs term is added
as a per-partition scalar (tensor_scalar) per 192-column block.
"""

import os
from contextlib import ExitStack

import numpy as np

# Problem constants (hardcoded per the harness contract).
H = 8
T = 10
EMB = 64
DIM_Q = 4708
DIM_K = 4708
DIM_I = 100
N_CAT = 10
DH, DW, DD = 24, 24, 8
NCONT = DH * DW * DD          # 4608 content rows/cols
PERIOD = DW * DD              # 192: column pattern period
NT = NCONT // 128             # 36 content row-tiles of 128
SCALE = float(EMB) ** -0.5    # 1/8
N_CORES = 8

_PROGRAM_CACHE = {}
LAST_RESULTS = None  # test harness introspection


def _build_program():
    """Build + compile the (shared, SPMD) Bass program once."""
    import concourse.tile as tile
    from concourse import bacc, mybir

    f32 = mybir.dt.float32
    nc = bacc.Bacc("TRN2", debug=False)

    cds_d = nc.dram_tensor("cds", [PERIOD, PERIOD], f32, kind="ExternalInput")
    svr_d = nc.dram_tensor("svr", [128, NT * DH], f32, kind="ExternalInput")
    cci_d = nc.dram_tensor("ccir", [128, DIM_I], f32, kind="ExternalInput")
    inst_d = nc.dram_tensor("inst", [DIM_I, DIM_I], f32, kind="ExternalInput")
    cic_d = nc.dram_tensor("cic", [DIM_I, 1], f32, kind="ExternalInput")
    out_d = nc.dram_tensor("out", [DIM_Q, DIM_K], f32, kind="ExternalOutput")

    with ExitStack() as ctx:
        tc = ctx.enter_context(tile.TileContext(nc))
        const = ctx.enter_context(tc.tile_pool(name="const", bufs=1))

        # Resident column pattern tiles: cdp[m][p, c] = cd[(128m+p) % 192, c],
        # c in [0, 192). Content row-tile t (rows 128t..128t+127) uses
        # cdp[t % 3]; every 192-column block of the output reads the SAME
        # [128, 192] source (the cd pattern repeats along columns), so no
        # expansion is needed. Loads are split across both HWDGE rings.
        cdp = [
            const.tile([128, PERIOD], f32, name=f"cdp{i}", tag=f"cdp{i}")
            for i in range(3)
        ]
        nc.sync.dma_start(cdp[0][:], cds_d[0:128])
        nc.sync.dma_start(cdp[1][0:64, :], cds_d[128:192])
        nc.sync.dma_start(cdp[1][64:128, :], cds_d[0:64])
        nc.sync.dma_start(cdp[2][:], cds_d[64:192])
        svr = const.tile([128, NT * DH], f32, tag="svr")
        nc.scalar.dma_start(svr[:], svr_d.ap())
        ccir = const.tile([128, DIM_I], f32, tag="ccir")
        nc.scalar.dma_start(ccir[:], cci_d.ap())
        inst_s = const.tile([DIM_I, DIM_I], f32, tag="inst")
        nc.scalar.dma_start(inst_s[:], inst_d.ap())
        cic_s = const.tile([DIM_I, 1], f32, tag="cic")
        nc.scalar.dma_start(cic_s[:], cic_d.ap())

        # Content rows [100:4708] in 36 tiles of 128 rows. Output stores
        # alternate between the two HWDGE rings (SP via nc.sync, ACT via
        # nc.scalar) to overlap per-DMA completion overheads.
        outp = ctx.enter_context(tc.tile_pool(name="outp", bufs=6))
        for t in range(NT):
            o = outp.tile([128, DIM_K], f32, tag="o")
            nc.vector.tensor_copy(o[:, :DIM_I], ccir[:])
            base = cdp[t % 3]
            for hj in range(DH):
                dst = o[:, DIM_I + PERIOD * hj : DIM_I + PERIOD * (hj + 1)]
                sv = svr[:, t * DH + hj : t * DH + hj + 1]
                if hj % 3 == 2:
                    nc.scalar.add(dst, base[:], sv)
                else:
                    nc.vector.tensor_scalar_add(dst, base[:], sv)
            dma_eng = nc.sync if t % 2 == 0 else nc.scalar
            dma_eng.dma_start(out_d[DIM_I + 128 * t : DIM_I + 128 * (t + 1), :], o[:])

        # Instruction rows [0:100] last (off the startup critical path).
        top = outp.tile([DIM_I, DIM_K], f32, tag="o", name="top")
        nc.vector.tensor_copy(top[:, :DIM_I], inst_s[:])
        for hj in range(DH):
            dst = top[:, DIM_I + PERIOD * hj : DIM_I + PERIOD * (hj + 1)]
            if hj % 3 == 2:
                # out = Identity(in * 0 + cic)
                nc.scalar.activation(
                    dst,
                    cdp[0][:DIM_I, :],
                    mybir.ActivationFunctionType.Identity,
                    bias=cic_s[:],
                    scale=0.0,
                )
            else:
                nc.vector.tensor_scalar(
                    dst,
                    cdp[0][:DIM_I, :],
                    0.0,
                    cic_s[:],
                    op0=mybir.AluOpType.mult,
                    op1=mybir.AluOpType.add,
                )
        nc.sync.dma_start(out_d[0:DIM_I, :], top[:])

    nc.compile()
    return nc


def _precompute(inputs):
    """Tiny per-head einsums in float64 -> compact fp32 device inputs."""
    f64 = np.float64
    g = {k: np.asarray(inputs[k], dtype=f64) for k in (
        "enc_intra", "enc_inter", "enc_cic", "enc_cci",
        "enc_h", "enc_w", "enc_d",
        "w_intra", "w_inter", "w_cic", "w_cci", "w_h", "w_w", "w_d",
    )}

    a_intra = np.einsum("hc,nmc->hnm", g["w_intra"], g["enc_intra"])  # [H,T,T]
    a_inter = np.einsum("hc,nmc->hnm", g["w_inter"], g["enc_inter"])
    intra_t = np.tile(a_intra, (1, N_CAT, N_CAT))                     # [H,100,100]
    inter_t = np.tile(a_inter, (1, N_CAT, N_CAT))
    mask = np.kron(np.eye(N_CAT, dtype=bool), np.ones((T, T), dtype=bool))
    inst = np.where(mask[None], intra_t, inter_t) * SCALE             # [H,100,100]

    cic = np.tile(
        np.einsum("hc,tc->ht", g["w_cic"], g["enc_cic"][:, 0, :]), (1, N_CAT)
    ) * SCALE                                                          # [H,100]
    cci = np.tile(
        np.einsum("hc,tc->ht", g["w_cci"], g["enc_cci"][0]), (1, N_CAT)
    ) * SCALE                                                          # [H,100]

    def rel_scores(w, table, n):
        b = np.einsum("hc,lc->hl", w, table)                 # [H, 2*cap-1]
        cap = (table.shape[0] + 1) // 2
        d = np.arange(n)[None, :] - np.arange(n)[:, None]
        idx = np.clip(d + cap - 1, 0, table.shape[0] - 1)
        return b[:, idx] * (SCALE / 3.0)                     # [H, n, n]

    rs = rel_scores(g["w_h"], g["enc_h"], DH)                # [H,24,24]
    cs = rel_scores(g["w_w"], g["enc_w"], DW)                # [H,24,24]
    ds = rel_scores(g["w_d"], g["enc_d"], DD)                # [H,8,8]

    # cd[h,a,b] = cs[h,a//8,b//8] + ds[h,a%8,b%8]  -> [H,192,192]
    cd = cs.repeat(DD, axis=1).repeat(DD, axis=2) + np.tile(ds, (1, DW, DW))
    cd32 = cd.astype(np.float32)

    # svr[h][p, t*24+hj] = rs[h, (128t+p)//192, hj]
    svec = np.repeat(rs, PERIOD, axis=1).astype(np.float32)  # [H,4608,24]
    svr = (
        svec.reshape(H, NT, 128, DH).transpose(0, 2, 1, 3).reshape(H, 128, NT * DH)
    )
    svr = np.ascontiguousarray(svr)

    cci_rep = np.ascontiguousarray(
        np.broadcast_to(cci[:, None, :], (H, 128, DIM_I))
    ).astype(np.float32)
    inst32 = inst.astype(np.float32)
    cic32 = cic.astype(np.float32)[:, :, None]

    in_maps = []
    for h in range(H):
        in_maps.append({
            "cds": np.ascontiguousarray(cd32[h]),
            "svr": svr[h],
            "ccir": cci_rep[h],
            "inst": np.ascontiguousarray(inst32[h]),
            "cic": np.ascontiguousarray(cic32[h]),
        })
    return in_maps


def kernel(**inputs):
    global LAST_RESULTS
    from concourse.bass_utils import run_bass_kernel_spmd

    assert int(inputs.get("dim_q", DIM_Q)) == DIM_Q
    assert int(inputs.get("dim_k", DIM_K)) == DIM_K
    assert int(inputs.get("dim_i", DIM_I)) == DIM_I
    assert int(inputs.get("dim_h", DH)) == DH
    assert int(inputs.get("dim_w", DW)) == DW
    assert int(inputs.get("dim_d", DD)) == DD

    if "nc" not in _PROGRAM_CACHE:
        _PROGRAM_CACHE["nc"] = _build_program()
    nc = _PROGRAM_CACHE["nc"]

    in_maps = _precompute(inputs)
    res = run_bass_kernel_spmd(
        nc,
        in_maps,
        core_ids=list(range(N_CORES)),
        tmpdir=os.environ.get("KERNEL_TRACE_DIR") or None,
    )
    LAST_RESULTS = res
    out = np.stack([res.results[c]["out"] for c in range(N_CORES)])
    return out[None]  # [1, H, Q, K]



# revision 8
# speedup vs baseline: 1.6180x; 1.6180x over previous
"""Trainium2 Bass kernel for InstructedAttentionPositionScores.

Computes the [1, H, Q, K] attention bias of the reference nn.Module.
Sharding: one head per NeuronCore (8 heads, 8 cores, tensor parallel).

Structure of the per-head [Q, K] output (Q = K = 4708, dim_i = 100):
  rows 0..99                       "instruction" rows
    cols 0..99   : inst block (block-diag intra/inter einsum scores)
    cols 100..   : cic[row] broadcast along columns
  rows 100..4707                   "content" rows (N = 24*24*8 = 4608)
    cols 0..99   : cci[col] broadcast along rows (every row identical)
    cols 100..   : content[i, j] = (rs[hi,hj] + cs[wi,wj] + ds[di,dj]) / 3
                   with i = hi*192 + wi*8 + di  (and same for j)

All einsums are tiny (<=10 MFLOP total) and are done on host in float64;
the device kernel does the memory-bound expansion to 8 x 88.7 MB:
  content[i, j] = cd[i % 192, j % 192] + rs[i // 192, j // 192]
where cd[a, b] = cs[a//8, b//8] + ds[a%8, b%8] is a [192, 192] pattern
kept resident in SBUF (replicated to a [384, 4608] tiled form so every
128-row output tile reads one aligned slice), and the rs term is added
as a per-partition scalar (tensor_scalar) per 192-column block.
"""

import os
from contextlib import ExitStack

import numpy as np

# Problem constants (hardcoded per the harness contract).
H = 8
T = 10
EMB = 64
DIM_Q = 4708
DIM_K = 4708
DIM_I = 100
N_CAT = 10
DH, DW, DD = 24, 24, 8
NCONT = DH * DW * DD          # 4608 content rows/cols
PERIOD = DW * DD              # 192: column pattern period
NT = NCONT // 128             # 36 content row-tiles of 128
SCALE = float(EMB) ** -0.5    # 1/8
N_CORES = 8

_PROGRAM_CACHE = {}
LAST_RESULTS = None  # test harness introspection


def _build_program():
    """Build + compile the (shared, SPMD) Bass program once."""
    import concourse.tile as tile
    from concourse import bacc, mybir

    f32 = mybir.dt.bfloat16  # device computes/stores bf16 (tolerance 2e-2)
    fsc = mybir.dt.float32   # per-partition scalar operands must be f32
    nc = bacc.Bacc("TRN2", debug=False)

    cds_d = nc.dram_tensor("cds", [PERIOD, PERIOD], f32, kind="ExternalInput")
    svr_d = nc.dram_tensor("svr", [128, NT * DH], fsc, kind="ExternalInput")
    cci_d = nc.dram_tensor("ccir", [128, DIM_I], f32, kind="ExternalInput")
    inst_d = nc.dram_tensor("inst", [DIM_I, DIM_I], f32, kind="ExternalInput")
    cic_d = nc.dram_tensor("cic", [DIM_I, 1], fsc, kind="ExternalInput")
    out_d = nc.dram_tensor("out", [DIM_Q, DIM_K], f32, kind="ExternalOutput")

    with ExitStack() as ctx:
        tc = ctx.enter_context(tile.TileContext(nc))
        const = ctx.enter_context(tc.tile_pool(name="const", bufs=1))

        # Resident column pattern tiles: cdp[m][p, c] = cd[(128m+p) % 192, c],
        # c in [0, 192). Content row-tile t (rows 128t..128t+127) uses
        # cdp[t % 3]; every 192-column block of the output reads the SAME
        # [128, 192] source (the cd pattern repeats along columns), so no
        # expansion is needed. Loads are split across both HWDGE rings.
        cdp = [
            const.tile([128, PERIOD], f32, name=f"cdp{i}", tag=f"cdp{i}")
            for i in range(3)
        ]
        nc.sync.dma_start(cdp[0][:], cds_d[0:128])
        nc.sync.dma_start(cdp[1][0:64, :], cds_d[128:192])
        nc.sync.dma_start(cdp[1][64:128, :], cds_d[0:64])
        nc.sync.dma_start(cdp[2][:], cds_d[64:192])
        svr = const.tile([128, NT * DH], fsc, tag="svr")
        nc.scalar.dma_start(svr[:], svr_d.ap())
        ccir = const.tile([128, DIM_I], f32, tag="ccir")
        nc.scalar.dma_start(ccir[:], cci_d.ap())
        inst_s = const.tile([DIM_I, DIM_I], f32, tag="inst")
        nc.scalar.dma_start(inst_s[:], inst_d.ap())
        cic_s = const.tile([DIM_I, 1], fsc, tag="cic")
        nc.scalar.dma_start(cic_s[:], cic_d.ap())

        # Content rows [100:4708] in 36 tiles of 128 rows. Output stores
        # alternate between the two HWDGE rings (SP via nc.sync, ACT via
        # nc.scalar) to overlap per-DMA completion overheads.
        outp = ctx.enter_context(tc.tile_pool(name="outp", bufs=6))
        for t in range(NT):
            o = outp.tile([128, DIM_K], f32, tag="o")
            nc.vector.tensor_copy(o[:, :DIM_I], ccir[:])
            base = cdp[t % 3]
            for hj in range(DH):
                dst = o[:, DIM_I + PERIOD * hj : DIM_I + PERIOD * (hj + 1)]
                sv = svr[:, t * DH + hj : t * DH + hj + 1]
                if hj % 3 == 2:
                    nc.scalar.add(dst, base[:], sv)
                else:
                    nc.vector.tensor_scalar_add(dst, base[:], sv)
            dma_eng = nc.sync if t % 2 == 0 else nc.scalar
            dma_eng.dma_start(out_d[DIM_I + 128 * t : DIM_I + 128 * (t + 1), :], o[:])

        # Instruction rows [0:100] last (off the startup critical path).
        top = outp.tile([DIM_I, DIM_K], f32, tag="o", name="top")
        nc.vector.tensor_copy(top[:, :DIM_I], inst_s[:])
        for hj in range(DH):
            dst = top[:, DIM_I + PERIOD * hj : DIM_I + PERIOD * (hj + 1)]
            if hj % 3 == 2:
                # out = Identity(in * 0 + cic)
                nc.scalar.activation(
                    dst,
                    cdp[0][:DIM_I, :],
                    mybir.ActivationFunctionType.Identity,
                    bias=cic_s[:],
                    scale=0.0,
                )
            else:
                nc.vector.tensor_scalar(
                    dst,
                    cdp[0][:DIM_I, :],
                    0.0,
                    cic_s[:],
                    op0=mybir.AluOpType.mult,
                    op1=mybir.AluOpType.add,
                )
        nc.sync.dma_start(out_d[0:DIM_I, :], top[:])

    nc.compile()
    return nc


def _precompute(inputs):
    """Tiny per-head einsums in float64 -> compact fp32 device inputs."""
    f64 = np.float64
    g = {k: np.asarray(inputs[k], dtype=f64) for k in (
        "enc_intra", "enc_inter", "enc_cic", "enc_cci",
        "enc_h", "enc_w", "enc_d",
        "w_intra", "w_inter", "w_cic", "w_cci", "w_h", "w_w", "w_d",
    )}

    a_intra = np.einsum("hc,nmc->hnm", g["w_intra"], g["enc_intra"])  # [H,T,T]
    a_inter = np.einsum("hc,nmc->hnm", g["w_inter"], g["enc_inter"])
    intra_t = np.tile(a_intra, (1, N_CAT, N_CAT))                     # [H,100,100]
    inter_t = np.tile(a_inter, (1, N_CAT, N_CAT))
    mask = np.kron(np.eye(N_CAT, dtype=bool), np.ones((T, T), dtype=bool))
    inst = np.where(mask[None], intra_t, inter_t) * SCALE             # [H,100,100]

    cic = np.tile(
        np.einsum("hc,tc->ht", g["w_cic"], g["enc_cic"][:, 0, :]), (1, N_CAT)
    ) * SCALE                                                          # [H,100]
    cci = np.tile(
        np.einsum("hc,tc->ht", g["w_cci"], g["enc_cci"][0]), (1, N_CAT)
    ) * SCALE                                                          # [H,100]

    def rel_scores(w, table, n):
        b = np.einsum("hc,lc->hl", w, table)                 # [H, 2*cap-1]
        cap = (table.shape[0] + 1) // 2
        d = np.arange(n)[None, :] - np.arange(n)[:, None]
        idx = np.clip(d + cap - 1, 0, table.shape[0] - 1)
        return b[:, idx] * (SCALE / 3.0)                     # [H, n, n]

    rs = rel_scores(g["w_h"], g["enc_h"], DH)                # [H,24,24]
    cs = rel_scores(g["w_w"], g["enc_w"], DW)                # [H,24,24]
    ds = rel_scores(g["w_d"], g["enc_d"], DD)                # [H,8,8]

    import ml_dtypes

    bf16 = ml_dtypes.bfloat16

    # cd[h,a,b] = cs[h,a//8,b//8] + ds[h,a%8,b%8]  -> [H,192,192]
    cd = cs.repeat(DD, axis=1).repeat(DD, axis=2) + np.tile(ds, (1, DW, DW))
    cd32 = cd.astype(bf16)

    # svr[h][p, t*24+hj] = rs[h, (128t+p)//192, hj]
    svec = np.repeat(rs, PERIOD, axis=1).astype(np.float32)  # [H,4608,24]
    svr = (
        svec.reshape(H, NT, 128, DH).transpose(0, 2, 1, 3).reshape(H, 128, NT * DH)
    )
    svr = np.ascontiguousarray(svr)

    cci_rep = np.ascontiguousarray(
        np.broadcast_to(cci[:, None, :], (H, 128, DIM_I))
    ).astype(bf16)
    inst32 = inst.astype(bf16)
    cic32 = cic.astype(np.float32)[:, :, None]

    in_maps = []
    for h in range(H):
        in_maps.append({
            "cds": np.ascontiguousarray(cd32[h]),
            "svr": svr[h],
            "ccir": cci_rep[h],
            "inst": np.ascontiguousarray(inst32[h]),
            "cic": np.ascontiguousarray(cic32[h]),
        })
    return in_maps


def kernel(**inputs):
    global LAST_RESULTS
    from concourse.bass_utils import run_bass_kernel_spmd

    assert int(inputs.get("dim_q", DIM_Q)) == DIM_Q
    assert int(inputs.get("dim_k", DIM_K)) == DIM_K
    assert int(inputs.get("dim_i", DIM_I)) == DIM_I
    assert int(inputs.get("dim_h", DH)) == DH
    assert int(inputs.get("dim_w", DW)) == DW
    assert int(inputs.get("dim_d", DD)) == DD

    if "nc" not in _PROGRAM_CACHE:
        _PROGRAM_CACHE["nc"] = _build_program()
    nc = _PROGRAM_CACHE["nc"]

    in_maps = _precompute(inputs)
    res = run_bass_kernel_spmd(
        nc,
        in_maps,
        core_ids=list(range(N_CORES)),
        tmpdir=os.environ.get("KERNEL_TRACE_DIR") or None,
    )
    LAST_RESULTS = res
    out = np.stack(
        [np.asarray(res.results[c]["out"], dtype=np.float32) for c in range(N_CORES)]
    )
    return out[None]  # [1, H, Q, K]



# revision 9
# speedup vs baseline: 1.8463x; 1.1411x over previous
"""Trainium2 Bass kernel for InstructedAttentionPositionScores.

Computes the [1, H, Q, K] attention bias of the reference nn.Module.
Sharding: one head per NeuronCore (8 heads, 8 cores, tensor parallel).

Structure of the per-head [Q, K] output (Q = K = 4708, dim_i = 100):
  rows 0..99                       "instruction" rows
    cols 0..99   : inst block (block-diag intra/inter einsum scores)
    cols 100..   : cic[row] broadcast along columns
  rows 100..4707                   "content" rows (N = 24*24*8 = 4608)
    cols 0..99   : cci[col] broadcast along rows (every row identical)
    cols 100..   : content[i, j] = (rs[hi,hj] + cs[wi,wj] + ds[di,dj]) / 3
                   with i = hi*192 + wi*8 + di  (and same for j)

All einsums are tiny (<=10 MFLOP total) and are done on host in float64;
the device kernel does the memory-bound expansion. The device works in
bf16 (output is cast back to f32 on host; tolerance is 2e-2, bf16 error
here is ~4e-3): halves HBM write traffic.

Each SBUF partition holds RPP consecutive output rows so each DMA
descriptor covers RPP*4708*2 contiguous DRAM bytes (bigger descriptors
amortize per-descriptor DMA-engine overhead):
  content[i, j] = cd[i % 192, j % 192] + rs[i // 192, j // 192]
where cd[a, b] = cs[a//8, b//8] + ds[a%8, b%8] is a [192, 192] pattern.
Pattern tiles cdp[o][p, s, c] = cd[(64*o + RPP*p + s) % 192, c] are host
precomputed for the 3 distinct row-phase offsets; the rs term is added
as a per-partition scalar (tensor_scalar) per 192-column block (the RPP
rows in a partition never straddle a 192-row block boundary).
"""

import os
from contextlib import ExitStack

import numpy as np

# Problem constants (hardcoded per the harness contract).
H = 8
T = 10
EMB = 64
DIM_Q = 4708
DIM_K = 4708
DIM_I = 100
N_CAT = 10
DH, DW, DD = 24, 24, 8
NCONT = DH * DW * DD          # 4608 content rows/cols
PERIOD = DW * DD              # 192: column pattern period
SCALE = float(EMB) ** -0.5    # 1/8
N_CORES = 8

RPP = 2                       # output rows packed per SBUF partition
TILE_ROWS = 128 * RPP         # content rows covered per tile
NT = NCONT // TILE_ROWS       # content tiles
assert NCONT % TILE_ROWS == 0

_PROGRAM_CACHE = {}
LAST_RESULTS = None  # test harness introspection


def _build_program():
    """Build + compile the (shared, SPMD) Bass program once."""
    import concourse.tile as tile
    from concourse import bacc, mybir

    bf = mybir.dt.bfloat16
    fsc = mybir.dt.float32   # per-partition scalar operands must be f32
    nc = bacc.Bacc("TRN2", debug=False)

    cds_d = nc.dram_tensor("cds", [3, 128, RPP * PERIOD], bf, kind="ExternalInput")
    svr_d = nc.dram_tensor("svr", [128, NT * DH], fsc, kind="ExternalInput")
    cci_d = nc.dram_tensor("ccir", [128, RPP * DIM_I], bf, kind="ExternalInput")
    inst_d = nc.dram_tensor("inst", [DIM_I, DIM_I], bf, kind="ExternalInput")
    cic_d = nc.dram_tensor("cic", [DIM_I, 1], fsc, kind="ExternalInput")
    out_d = nc.dram_tensor("out", [DIM_Q * DIM_K], bf, kind="ExternalOutput")

    with ExitStack() as ctx:
        tc = ctx.enter_context(tile.TileContext(nc))
        const = ctx.enter_context(tc.tile_pool(name="const", bufs=1))

        # Resident column-pattern tiles, one per row-phase offset; loads are
        # split across both HWDGE rings.
        cdp = []
        for i in range(3):
            t_ = const.tile([128, RPP, PERIOD], bf, name=f"cdp{i}", tag=f"cdp{i}")
            eng = nc.sync if i % 2 == 0 else nc.scalar
            eng.dma_start(t_[:], cds_d[i])
            cdp.append(t_)
        svr = const.tile([128, NT * DH], fsc, tag="svr")
        nc.scalar.dma_start(svr[:], svr_d.ap())
        ccir = const.tile([128, RPP, DIM_I], bf, tag="ccir")
        nc.sync.dma_start(ccir[:], cci_d.ap())
        inst_s = const.tile([DIM_I, DIM_I], bf, tag="inst")
        nc.scalar.dma_start(inst_s[:], inst_d.ap())
        cic_s = const.tile([DIM_I, 1], fsc, tag="cic")
        nc.sync.dma_start(cic_s[:], cic_d.ap())

        # Content rows [100:4708] in NT tiles of TILE_ROWS rows; partition p
        # of tile t holds output rows 100 + TILE_ROWS*t + RPP*p .. +RPP-1.
        # Output stores alternate between the two HWDGE rings.
        outp = ctx.enter_context(tc.tile_pool(name="outp", bufs=6))
        for t in range(NT):
            o = outp.tile([128, RPP, DIM_K], bf, tag="o")
            nc.vector.tensor_copy(o[:, :, :DIM_I], ccir[:])
            base = cdp[t % 3]
            for hj in range(DH):
                dst = o[:, :, DIM_I + PERIOD * hj : DIM_I + PERIOD * (hj + 1)]
                sv = svr[:, t * DH + hj : t * DH + hj + 1]
                if hj % 3 == 2:
                    nc.scalar.add(dst, base[:], sv)
                else:
                    nc.vector.tensor_scalar_add(dst, base[:], sv)
            dma_eng = nc.sync if t % 2 == 0 else nc.scalar
            flat0 = (DIM_I + TILE_ROWS * t) * DIM_K
            dma_eng.dma_start(out_d[flat0 : flat0 + TILE_ROWS * DIM_K], o[:])

        # Instruction rows [0:100] last (off the startup critical path).
        top = outp.tile([DIM_I, DIM_K], bf, tag="o", name="top")
        nc.vector.tensor_copy(top[:, :DIM_I], inst_s[:])
        for hj in range(DH):
            dst = top[:, DIM_I + PERIOD * hj : DIM_I + PERIOD * (hj + 1)]
            if hj % 3 == 2:
                # out = Identity(in * 0 + cic)
                nc.scalar.activation(
                    dst,
                    cdp[0][:DIM_I, 0, :],
                    mybir.ActivationFunctionType.Identity,
                    bias=cic_s[:],
                    scale=0.0,
                )
            else:
                nc.vector.tensor_scalar(
                    dst,
                    cdp[0][:DIM_I, 0, :],
                    0.0,
                    cic_s[:],
                    op0=mybir.AluOpType.mult,
                    op1=mybir.AluOpType.add,
                )
        nc.sync.dma_start(out_d[0 : DIM_I * DIM_K], top[:])

    nc.compile()
    return nc


def _precompute(inputs):
    """Tiny per-head einsums in float64 -> compact device inputs."""
    import ml_dtypes

    bf16 = ml_dtypes.bfloat16
    f64 = np.float64
    g = {k: np.asarray(inputs[k], dtype=f64) for k in (
        "enc_intra", "enc_inter", "enc_cic", "enc_cci",
        "enc_h", "enc_w", "enc_d",
        "w_intra", "w_inter", "w_cic", "w_cci", "w_h", "w_w", "w_d",
    )}

    a_intra = np.einsum("hc,nmc->hnm", g["w_intra"], g["enc_intra"])  # [H,T,T]
    a_inter = np.einsum("hc,nmc->hnm", g["w_inter"], g["enc_inter"])
    intra_t = np.tile(a_intra, (1, N_CAT, N_CAT))                     # [H,100,100]
    inter_t = np.tile(a_inter, (1, N_CAT, N_CAT))
    mask = np.kron(np.eye(N_CAT, dtype=bool), np.ones((T, T), dtype=bool))
    inst = np.where(mask[None], intra_t, inter_t) * SCALE             # [H,100,100]

    cic = np.tile(
        np.einsum("hc,tc->ht", g["w_cic"], g["enc_cic"][:, 0, :]), (1, N_CAT)
    ) * SCALE                                                          # [H,100]
    cci = np.tile(
        np.einsum("hc,tc->ht", g["w_cci"], g["enc_cci"][0]), (1, N_CAT)
    ) * SCALE                                                          # [H,100]

    def rel_scores(w, table, n):
        b = np.einsum("hc,lc->hl", w, table)                 # [H, 2*cap-1]
        cap = (table.shape[0] + 1) // 2
        d = np.arange(n)[None, :] - np.arange(n)[:, None]
        idx = np.clip(d + cap - 1, 0, table.shape[0] - 1)
        return b[:, idx] * (SCALE / 3.0)                     # [H, n, n]

    rs = rel_scores(g["w_h"], g["enc_h"], DH)                # [H,24,24]
    cs = rel_scores(g["w_w"], g["enc_w"], DW)                # [H,24,24]
    ds = rel_scores(g["w_d"], g["enc_d"], DD)                # [H,8,8]

    # cd[h,a,b] = cs[h,a//8,b//8] + ds[h,a%8,b%8]  -> [H,192,192]
    cd = cs.repeat(DD, axis=1).repeat(DD, axis=2) + np.tile(ds, (1, DW, DW))

    # cds[h][i, p, s*192+c] = cd[h, (64*i + RPP*p + s) % 192, c]
    offs = (TILE_ROWS * np.arange(3)) % PERIOD               # row-phase offsets
    p_idx = np.arange(128)
    s_idx = np.arange(RPP)
    rows = (offs[:, None, None] + RPP * p_idx[None, :, None]
            + s_idx[None, None, :]) % PERIOD                 # [3,128,RPP]
    cds = cd[:, rows, :].reshape(H, 3, 128, RPP * PERIOD).astype(bf16)

    # svr[h][p, t*24+hj] = rs[h, (TILE_ROWS*t + RPP*p)//192, hj]
    hi = (TILE_ROWS * np.arange(NT)[:, None] + RPP * p_idx[None, :]) // PERIOD
    svr = rs[:, hi, :]                                       # [H,NT,128,24]
    svr = (
        svr.transpose(0, 2, 1, 3).reshape(H, 128, NT * DH).astype(np.float32)
    )
    svr = np.ascontiguousarray(svr)

    cci_rep = np.ascontiguousarray(
        np.broadcast_to(cci[:, None, :], (H, 128 * RPP, DIM_I))
    ).reshape(H, 128, RPP * DIM_I).astype(bf16)
    inst16 = inst.astype(bf16)
    cic32 = cic.astype(np.float32)[:, :, None]

    in_maps = []
    for h in range(H):
        in_maps.append({
            "cds": np.ascontiguousarray(cds[h]),
            "svr": svr[h],
            "ccir": cci_rep[h],
            "inst": np.ascontiguousarray(inst16[h]),
            "cic": np.ascontiguousarray(cic32[h]),
        })
    return in_maps


def kernel(**inputs):
    global LAST_RESULTS
    from concourse.bass_utils import run_bass_kernel_spmd

    assert int(inputs.get("dim_q", DIM_Q)) == DIM_Q
    assert int(inputs.get("dim_k", DIM_K)) == DIM_K
    assert int(inputs.get("dim_i", DIM_I)) == DIM_I
    assert int(inputs.get("dim_h", DH)) == DH
    assert int(inputs.get("dim_w", DW)) == DW
    assert int(inputs.get("dim_d", DD)) == DD

    if "nc" not in _PROGRAM_CACHE:
        _PROGRAM_CACHE["nc"] = _build_program()
    nc = _PROGRAM_CACHE["nc"]

    in_maps = _precompute(inputs)
    res = run_bass_kernel_spmd(
        nc,
        in_maps,
        core_ids=list(range(N_CORES)),
        tmpdir=os.environ.get("KERNEL_TRACE_DIR") or None,
    )
    LAST_RESULTS = res
    out = np.stack(
        [
            np.asarray(res.results[c]["out"], dtype=np.float32).reshape(DIM_Q, DIM_K)
            for c in range(N_CORES)
        ]
    )
    return out[None]  # [1, H, Q, K]
